# revision 14
# baseline (speedup 1.0000x reference)
"""Trainium2 Bass kernel for nn_DeformConv2d (B=8, H=W=128, C=192, G=6, K=3).

Data-parallel over batch: one image per NeuronCore (8 cores).

v2: tiny deformable offsets make bilinear sampling an exact 5x5 stencil with
data-dependent per-(position, group) weights S.  Everything heavy runs in
bf16 (5.8e-3 rel err vs the 2e-2 gate).

Per chunk of 16 rows (channel-major front-end, w-major apply):
  - load x (bf16, host-converted), PE-transpose to channel-major
  - xp = x @ w_in (bf16 GEMM, Act adds bias -> bf16, replicate-pad in w)
  - depthwise 3x3 as diagonal bf16 matmuls + fused Act Silu
  - offsets/mask GEMM with host-permuted (tap, group)-ordered columns;
    PE-transpose to w-major; softmax; branch-free bilinear weights;
    27 strided adds build the 25-bin stencil S[w, t, bin, g] bf16
  - S duplicated to channel pairs S2[w, t, bin, g, 2] (SBUF->SBUF DMA) so
    apply multiplies hit DVE 2x_1P (stride-0 group broadcast with a
    unit-stride innermost pair)
  - 5 dx-shifted window PE-transposes of padded xp -> xq[dx] (the window into
    the replicate pad encodes the W-border clamp exactly)
  - apply: acc[w, t, c] += S2[bin] * xq[dx][t+dy]; dx=-2 bin group runs on
    Pool with its own accumulator, rest on DVE; merged at the end
  - transpose acc to channel-major, out = acc @ w_out + b_out, transpose
    back to position-major fp32, DMA out.
"""

import sys

import numpy as np

sys.path.insert(0, "/opt/trn_rl_repo")

B, H, W, C = 8, 128, 128, 192
G, K = 6, 3
K2 = K * K
GC = C // G
OFFSET_SCALE = 0.1
POS = H * W

NCORES = 8
RCH = 16             # rows per chunk
NCH = H // RCH
NBUF = RCH + 4       # buffered rows (+-2 halo)
WP = W + 4           # replicate-padded width for xp
XST = 132            # x_cm row stride (128 data + 4 zero pad)

# bins grouped by dx so each xq[dx] buffer drains in order; dx=-2 group
# (indices 0-4) runs on Pool
BINS = [(dy, dx) for dx in range(-2, 3) for dy in range(-2, 3)]

_CACHE = {}


def _bf(a):
    import ml_dtypes
    return np.ascontiguousarray(np.asarray(a, np.float32)
                                .astype(ml_dtypes.bfloat16))


def _host_weights(inp):
    f = lambda a: np.ascontiguousarray(np.asarray(a, dtype=np.float32))
    w_in = f(inp["w_in"]); b_in = f(inp["b_in"])
    w_dw = f(inp["w_dw"]); b_dw = f(inp["b_dw"])
    w_pw = f(inp["w_pw"]).reshape(C, C); b_pw = f(inp["b_pw"])
    w_off = f(inp["w_off"]); b_off = f(inp["b_off"])
    w_mask = f(inp["w_mask"]); b_mask = f(inp["b_mask"])
    w_out = f(inp["w_out"]); b_out = f(inp["b_out"])

    w_off2 = w_pw @ w_off
    b_off2 = b_pw @ w_off + b_off
    w_msk2 = w_pw @ w_mask
    b_msk2 = b_pw @ w_mask + b_mask

    # permute 54-wide blocks from (g, tap) to (tap, g) order
    perm = np.array([g * K2 + t for t in range(K2) for g in range(G)])
    oh_cols = w_off2[:, 0::2][:, perm] * OFFSET_SCALE
    ow_cols = w_off2[:, 1::2][:, perm] * OFFSET_SCALE
    mk_cols = w_msk2[:, perm]
    b_oh = b_off2[0::2][perm] * OFFSET_SCALE
    b_ow = b_off2[1::2][perm] * OFFSET_SCALE
    b_mk = b_msk2[perm]
    w_om = np.concatenate([oh_cols, ow_cols, mk_cols], axis=1)
    b_om = np.concatenate([b_oh, b_ow, b_mk])

    dwd0 = np.zeros((128, 9, 128), np.float32)
    dwd1 = np.zeros((64, 9, 64), np.float32)
    for k in range(9):
        d = w_dw[k // 3, k % 3, 0, :]
        np.fill_diagonal(dwd0[:, k, :], d[0:128])
        np.fill_diagonal(dwd1[:, k, :], d[128:192])

    return {
        "wiA": _bf(w_in[0:128, :]), "wiB": _bf(w_in[128:192, :]),
        "binA": f(b_in[0:128].reshape(128, 1)),
        "binB": f(b_in[128:192].reshape(64, 1)),
        "dwd0": _bf(dwd0), "dwd1": _bf(dwd1),
        "bdwA": f(b_dw[0:128].reshape(128, 1)),
        "bdwB": f(b_dw[128:192].reshape(64, 1)),
        "womA": _bf(w_om[0:128, :]), "womB": _bf(w_om[128:192, :]),
        "bomA": f(b_om[0:128].reshape(128, 1)),
        "bomB": f(b_om[128:162].reshape(34, 1)),
        "woA": _bf(w_out[0:128, :]), "woB": _bf(w_out[128:192, :]),
        "boA": f(b_out[0:128].reshape(128, 1)),
        "boB": f(b_out[128:192].reshape(64, 1)),
        "idnb": _bf(np.eye(128, dtype=np.float32)),
        "idnr": np.ascontiguousarray(np.eye(128, dtype=np.float32)),
    }


WSHAPES = [
    ("wiA", [128, C], "bf16"), ("wiB", [64, C], "bf16"),
    ("binA", [128, 1], "f32"), ("binB", [64, 1], "f32"),
    ("dwd0", [128, 9, 128], "bf16"), ("dwd1", [64, 9, 64], "bf16"),
    ("bdwA", [128, 1], "f32"), ("bdwB", [64, 1], "f32"),
    ("womA", [128, 162], "bf16"), ("womB", [64, 162], "bf16"),
    ("bomA", [128, 1], "f32"), ("bomB", [34, 1], "f32"),
    ("woA", [128, C], "bf16"), ("woB", [64, C], "bf16"),
    ("boA", [128, 1], "f32"), ("boB", [64, 1], "f32"),
    ("idnb", [128, 128], "bf16"), ("idnr", [128, 128], "f32r"),
]


def build_program(npass=1):
    key = ("nc", npass)
    if key in _CACHE:
        return _CACHE[key]

    import concourse.bacc as bacc
    import concourse.tile as tile
    import concourse.mybir as mybir

    F32 = mybir.dt.float32
    F32R = mybir.dt.float32r
    BF16 = mybir.dt.bfloat16
    OP = mybir.AluOpType
    AF = mybir.ActivationFunctionType
    AX = mybir.AxisListType

    nc = bacc.Bacc(None, target_bir_lowering=False)

    x_d = nc.dram_tensor("x", [POS, C], BF16, kind="ExternalInput")
    out_d = nc.dram_tensor("out", [POS, C], F32, kind="ExternalOutput")
    DT = {"f32": F32, "bf16": BF16, "f32r": F32R}
    wd = {name: nc.dram_tensor(name, shape, DT[dts], kind="ExternalInput")
          for name, shape, dts in WSHAPES}

    x_dv = x_d[:].rearrange("(h p) c -> p h c", p=W)
    out_dv = out_d[:].rearrange("(h p) c -> p h c", p=W)

    with tile.TileContext(nc) as tc:
        with (
            tc.tile_pool(name="wp", bufs=1) as wp,
            tc.tile_pool(name="st1", bufs=1) as st1,
            tc.tile_pool(name="st2", bufs=2) as st2,
            tc.tile_pool(name="ps", bufs=3, space="PSUM") as ps,
            tc.tile_pool(name="psq", bufs=2, space="PSUM") as psq,
            tc.tile_pool(name="pso", bufs=2, space="PSUM") as pso,
        ):
            w = {}
            for name, shape, dts in WSHAPES:
                w[name] = wp.tile(list(shape), DT[dts], tag=name,
                                  name="w_" + name)
                nc.sync.dma_start(w[name][:], wd[name][:])

            state = {}

            def front_end(ci):
                h0 = ci * RCH

                # ---- load + transpose x to channel-major (bf16) ----
                x_cmA = st1.tile([128, NBUF, XST], BF16, tag="x_cmA")
                x_cmB = st1.tile([64, NBUF, XST], BF16, tag="x_cmB")
                for nb in range(5):
                    xt = st2.tile([W, 4, C], BF16, tag="x_pm", bufs=2)
                    rows = [min(max(h0 - 2 + 4 * nb + j, 0), H - 1)
                            for j in range(4)]
                    j = 0
                    while j < 4:
                        j2 = j
                        while j2 + 1 < 4 and rows[j2 + 1] == rows[j2] + 1:
                            j2 += 1
                        nc.sync.dma_start(xt[:, j:j2 + 1, :],
                                          x_dv[:, rows[j]:rows[j2] + 1, :])
                        j = j2 + 1
                    pt = ps.tile([128, 4, 256], BF16, tag="mm", name="ptx")
                    for jr in range(4):
                        nc.tensor.transpose(pt[:, jr, 0:128],
                                            xt[:, jr, 0:128], w["idnb"][:])
                        nc.tensor.transpose(pt[0:64, jr, 128:256],
                                            xt[:, jr, 128:192], w["idnb"][:])
                    r0 = 4 * nb
                    nc.scalar.copy(x_cmA[:, r0:r0 + 4, 0:128],
                                   pt[:, :, 0:128])
                    nc.scalar.copy(x_cmB[:, r0:r0 + 4, 0:128],
                                   pt[0:64, :, 128:256])
                nc.vector.memset(x_cmA[:, :, 128:132], 0.0)
                nc.vector.memset(x_cmB[:, :, 128:132], 0.0)

                # ---- xp = x @ w_in -> padded bf16 buffers ----
                xpA = st1.tile([128, NBUF, WP], BF16, tag="xpA", bufs=2)
                xpB = st1.tile([64, NBUF, WP], BF16, tag="xpB", bufs=2)
                xA_f = x_cmA[:].rearrange("p a b -> p (a b)")
                xB_f = x_cmB[:].rearrange("p a b -> p (a b)")
                ABLK = [(0, 3), (3, 6), (6, 9), (9, 12), (12, 15), (15, 18),
                        (18, 20)]
                for r0, r1 in ABLK:
                    nr = r1 - r0
                    pa = ps.tile([128, 512], F32, tag="mm", name="pa")
                    nc.tensor.matmul(pa[:, 0:XST * nr], w["wiA"][:, 0:128],
                                     xA_f[:, XST * r0:XST * r1],
                                     start=True, stop=False)
                    nc.tensor.matmul(pa[:, 0:XST * nr], w["wiB"][:, 0:128],
                                     xB_f[:, XST * r0:XST * r1],
                                     start=False, stop=True)
                    pav = pa[:, 0:XST * nr].rearrange("p (r w) -> p r w", r=nr)
                    nc.scalar.activation(xpA[:, r0:r1, 2:130],
                                         pav[:, :, 0:128], AF.Identity,
                                         bias=w["binA"][:], scale=1.0)
                    pb = ps.tile([128, 512], F32, tag="mm", name="pb")
                    nc.tensor.matmul(pb[0:64, 0:XST * nr],
                                     w["wiA"][:, 128:192],
                                     xA_f[:, XST * r0:XST * r1],
                                     start=True, stop=False)
                    nc.tensor.matmul(pb[0:64, 0:XST * nr],
                                     w["wiB"][:, 128:192],
                                     xB_f[:, XST * r0:XST * r1],
                                     start=False, stop=True)
                    pbv = pb[0:64, 0:XST * nr].rearrange("p (r w) -> p r w",
                                                         r=nr)
                    nc.scalar.activation(xpB[:, r0:r1, 2:130],
                                         pbv[:, :, 0:128], AF.Identity,
                                         bias=w["binB"][:], scale=1.0)
                for t_, np_ in ((xpA, 128), (xpB, 64)):
                    nc.vector.tensor_copy(
                        t_[:, :, 0:2],
                        t_[:, :, 2:3].broadcast_to([np_, NBUF, 2]))
                    nc.vector.tensor_copy(
                        t_[:, :, 130:132],
                        t_[:, :, 129:130].broadcast_to([np_, NBUF, 2]))

                # ---- depthwise conv + fused SiLU ----
                sA = st1.tile([128, RCH, W], BF16, tag="sA")
                sB = st1.tile([64, RCH, W], BF16, tag="sB")
                taps = [(0, -1), (0, 0), (0, 1), (-1, -1), (-1, 0), (-1, 1),
                        (1, -1), (1, 0), (1, 1)]
                DBLK = [(0, 3), (3, 6), (6, 9), (9, 12), (12, 14), (14, 16)]
                for dwt, cmf, st_, bdw, npart in (
                        ("dwd0", xA_f, sA, "bdwA", 128),
                        ("dwd1", xB_f, sB, "bdwB", 64)):
                    for r0, r1 in DBLK:
                        nr = r1 - r0
                        pd = ps.tile([128, 512], F32, tag="mm", name="pd")
                        issued = 0
                        for ti, (dy, dx) in enumerate(taps):
                            rl, rh_ = r0, r1
                            if ci == 0 and dy == -1:
                                rl = max(rl, 1)
                            if ci == NCH - 1 and dy == 1:
                                rh_ = min(rh_, RCH - 1)
                            if rl >= rh_:
                                continue
                            base = XST * (rl + 2 + dy) + dx
                            nc.tensor.matmul(
                                pd[0:npart, XST * (rl - r0):XST * (rh_ - r0)],
                                w[dwt][:, (dy + 1) * 3 + (dx + 1), :],
                                cmf[:, base:base + XST * (rh_ - rl)],
                                start=(issued == 0),
                                stop=(ti == len(taps) - 1),
                                skip_group_check=True)
                            issued += 1
                        pdv = pd[0:npart, 0:XST * nr].rearrange(
                            "p (r w) -> p r w", r=nr)[:, :, 0:128]
                        nc.scalar.activation(st_[:, r0:r1, :], pdv, AF.Silu,
                                             bias=w[bdw][:], scale=1.0)

                # ---- offsets/mask projection + transpose to w-major ----
                # columns: [oh(54) | ow(54) | mask(54)], each (tap, g)-major
                ohow = st1.tile([W, RCH, 108], BF16, tag="ohow")
                expm = st1.tile([W, RCH, 54], BF16, tag="expm")
                for nb in range(4):
                    rsl = slice(4 * nb, 4 * nb + 4)
                    omA = st2.tile([128, 4, W], BF16, tag="omA", bufs=2)
                    omB = st2.tile([34, 4, W], BF16, tag="omB", bufs=2)
                    for msl, omt, npart, bom in (
                            (slice(0, 128), omA, 128, "bomA"),
                            (slice(128, 162), omB, 34, "bomB")):
                        po = ps.tile([128, 512], F32, tag="mm", name="pom")
                        pov = po[0:npart, :].rearrange("p (r w) -> p r w", r=4)
                        nc.tensor.matmul(
                            po[0:npart, :], w["womA"][:, msl],
                            sA[:, rsl, :].rearrange("p a b -> p (a b)"),
                            start=True, stop=False)
                        nc.tensor.matmul(
                            po[0:npart, :], w["womB"][:, msl],
                            sB[:, rsl, :].rearrange("p a b -> p (a b)"),
                            start=False, stop=True)
                        nc.scalar.activation(omt[:], pov, AF.Identity,
                                             bias=w[bom][:], scale=1.0)
                    pt = psq.tile([128, 4, 162], BF16, tag="tq", name="ptom")
                    for jt in range(4):
                        nc.tensor.transpose(pt[:, jt, 0:128], omA[:, jt, :],
                                            w["idnb"][:])
                        nc.tensor.transpose(pt[:, jt, 128:162], omB[:, jt, :],
                                            w["idnb"][0:34, 0:34])
                    t0 = 4 * nb
                    nc.scalar.copy(ohow[:, t0:t0 + 4, :], pt[:, :, 0:108])
                    nc.scalar.activation(expm[:, t0:t0 + 4, :],
                                         pt[:, :, 108:162], AF.Exp)

                # ---- softmax over taps (tap-major: expm[w, t, 9, 6]) ----
                red = st2.tile([W, RCH, 6], F32, tag="red", bufs=2)
                nc.vector.tensor_reduce(
                    red[:],
                    expm[:].rearrange("p t (k g) -> p t g k", g=6),
                    AX.X, OP.add)
                rec = st2.tile([W, RCH, 6], BF16, tag="rec", bufs=2)
                with nc.allow_low_precision(reason="attn recip in bf16"):
                    nc.vector.reciprocal(rec[:], red[:])
                attn = st1.tile([W, RCH, 54], BF16, tag="attn")
                nc.vector.tensor_tensor(
                    attn[:].rearrange("p t (k g) -> p t k g", g=6),
                    expm[:].rearrange("p t (k g) -> p t k g", g=6),
                    rec[:].unsqueeze(2).broadcast_to([W, RCH, 9, 6]),
                    OP.mult)

                # ---- branch-free bilinear weights ----
                oh_v = ohow[:, :, 0:54]
                ow_v = ohow[:, :, 54:108]
                hwm = st1.tile([W, RCH, 54], BF16, tag="hwm")
                hwp = st1.tile([W, RCH, 54], BF16, tag="hwp")
                wwm = st1.tile([W, RCH, 54], BF16, tag="wwm")
                wwp = st1.tile([W, RCH, 54], BF16, tag="wwp")
                nc.scalar.activation(hwm[:], oh_v, AF.Relu, bias=0.0,
                                     scale=-1.0)
                nc.scalar.activation(hwp[:], oh_v, AF.Relu)
                nc.scalar.activation(wwm[:], ow_v, AF.Relu, bias=0.0,
                                     scale=-1.0)
                nc.scalar.activation(wwp[:], ow_v, AF.Relu)
                # negated centers: |o| - 1 = -(1 - |o|)
                ww0n = st1.tile([W, RCH, 54], BF16, tag="ww0n")
                nc.scalar.activation(ww0n[:], ow_v, AF.Abs)
                nc.vector.tensor_scalar_sub(ww0n[:], ww0n[:], 1.0)
                t1h = st2.tile([W, RCH, 54], BF16, tag="t1h", bufs=2)
                nc.scalar.activation(t1h[:], oh_v, AF.Abs)
                nc.vector.tensor_scalar_sub(t1h[:], t1h[:], 1.0)
                ahm = st1.tile([W, RCH, 54], BF16, tag="ahm")
                ahp = st1.tile([W, RCH, 54], BF16, tag="ahp")
                ah0n = st1.tile([W, RCH, 54], BF16, tag="ah0n")
                nc.vector.tensor_tensor(ahm[:], attn[:], hwm[:], OP.mult)
                nc.vector.tensor_tensor(ahp[:], attn[:], hwp[:], OP.mult)
                nc.vector.tensor_tensor(ah0n[:], attn[:], t1h[:], OP.mult)

                # ---- accumulate 25-bin stencil S[w, t, 25, 6] (bf16) ----
                S = st1.tile([W, 25, RCH, 6], BF16, tag="S")
                nc.vector.memset(S[:], 0.0)
                ah = {-1: (ahm, 1), 0: (ah0n, -1), 1: (ahp, 1)}
                ww = {-1: (wwm, 1), 0: (ww0n, -1), 1: (wwp, 1)}
                for a in (-1, 0, 1):
                    ah_t, sgn_a = ah[a]
                    for b_ in (-1, 0, 1):
                        ww_t, sgn_b = ww[b_]
                        pab = st2.tile([W, RCH, 54], BF16, tag="pab", bufs=3)
                        nc.vector.tensor_tensor(pab[:], ah_t[:], ww_t[:],
                                                OP.mult)
                        op = OP.add if sgn_a * sgn_b > 0 else OP.subtract
                        pv = pab[:].rearrange("p t (rh rw g) -> p rh rw t g",
                                              rh=3, rw=3)
                        for rh_ in range(3):
                            dy5 = rh_ - 1 + a + 2
                            tgt = (S[:]
                                   .rearrange("p (dy dx) t g -> p dy dx t g",
                                              dy=5)
                                   [:, dy5, b_ + 1:b_ + 4, :, :])
                            nc.vector.tensor_tensor(
                                tgt, tgt, pv[:, rh_, :, :, :], op)

                # ---- duplicate S -> S2 channel pairs (SBUF->SBUF DMA) ----
                S2 = st1.tile([W, 25, RCH, 6, 2], BF16, tag="S2", bufs=2)
                nc.vector.tensor_copy(
                    S2[:],
                    S[:].unsqueeze(4).broadcast_to([W, 25, RCH, 6, 2]))

                state[ci] = {"xpA": xpA, "xpB": xpB, "S2": S2, "xq": None, "accd": None}

            def xq_transposes(ci):
                stc = state[ci]
                xpA, xpB = stc["xpA"], stc["xpB"]
                xq = {}
                xq[0] = st1.tile([128, NBUF, C], BF16, tag="xq0",
                                 name="xq0")
                for g4 in range(5):
                    pt = psq.tile([128, 4, 192], BF16, tag="tq", name="ptq")
                    for jr in range(4):
                        r = 4 * g4 + jr
                        nc.tensor.transpose(pt[:, jr, 0:128],
                                            xpA[:, r, 2:130], w["idnb"][:])
                        nc.tensor.transpose(pt[:, jr, 128:192],
                                            xpB[:, r, 2:130],
                                            w["idnb"][0:64, 0:64])
                    if g4 % 2 == 0:
                        nc.vector.tensor_copy(
                            xq[0][:, 4 * g4:4 * g4 + 4, :], pt[:])
                    else:
                        nc.scalar.copy(
                            xq[0][:, 4 * g4:4 * g4 + 4, :], pt[:])
                # partition-shifted copies (W-border clamp via edge dup)
                for dx in (-2, -1, 1, 2):
                    t = st1.tile([128, NBUF, C], BF16, tag=f"xq{dx}",
                                 name=f"xq{dx}")
                    if dx > 0:
                        nc.sync.dma_start(t[0:128 - dx], xq[0][dx:128])
                        for e in range(dx):
                            nc.sync.dma_start(t[128 - dx + e:129 - dx + e],
                                              xq[0][127:128])
                    else:
                        d = -dx
                        nc.sync.dma_start(t[d:128], xq[0][0:128 - d])
                        for e in range(d):
                            nc.sync.dma_start(t[e:e + 1], xq[0][0:1])
                    xq[dx] = t
                stc["xq"] = xq

            def bin_views(stc, i):
                dy, dx = BINS[i]
                si = (dy + 2) * 5 + (dx + 2)
                xv = (stc["xq"][dx][:, 2 + dy:2 + dy + RCH, :]
                      .rearrange("p t (g h two) -> p t g h two", g=6, two=2))
                sv = (stc["S2"][:, si, :, :, :].unsqueeze(3)
                      .broadcast_to([W, RCH, 6, GC // 2, 2]))
                return xv, sv


            def apply_dve(ci, lo, hi):
                stc = state[ci]
                accd = stc["accd"]
                if accd is None:
                    accd = st1.tile([128, RCH, C], BF16, tag="accd", bufs=2,
                                    name="accd")
                    stc["accd"] = accd
                for i in range(lo, hi):
                    xv, sv = bin_views(stc, i)
                    if i == 0:
                        av = accd[:].rearrange("p t (g h two) -> p t g h two",
                                               g=6, two=2)
                        nc.vector.tensor_tensor(av, xv, sv, OP.mult)
                    else:
                        tmp = st2.tile([128, RCH, C], BF16, tag="tmpA",
                                       bufs=2, name="tmpA")
                        tv = tmp[:].rearrange("p t (g h two) -> p t g h two",
                                              g=6, two=2)
                        nc.vector.tensor_tensor(tv, xv, sv, OP.mult)
                        nc.vector.tensor_tensor(accd[:], accd[:], tmp[:],
                                                OP.add)

            def finish(ci):
                h0 = ci * RCH
                stc = state.pop(ci)
                accd = stc["accd"]

                # ---- transpose acc to channel-major ----
                acmA = st2.tile([128, RCH, W], BF16, tag="acmA", bufs=1)
                acmB = st2.tile([64, RCH, W], BF16, tag="acmB", bufs=1)
                for g4 in range(4):
                    pc = pso.tile([128, 4, 256], BF16, tag="pc")
                    for jt in range(4):
                        t = 4 * g4 + jt
                        nc.tensor.transpose(pc[:, jt, 0:128],
                                            accd[:, t, 0:128], w["idnb"][:])
                        nc.tensor.transpose(pc[0:64, jt, 128:256],
                                            accd[:, t, 128:192],
                                            w["idnb"][:])
                    t0 = 4 * g4
                    nc.scalar.copy(acmA[:, t0:t0 + 4, :], pc[:, :, 0:128])
                    nc.vector.tensor_copy(acmB[:, t0:t0 + 4, :],
                                          pc[0:64, :, 128:256])

                # ---- out projection + transpose back + store ----
                for g4 in range(4):
                    qs = slice(4 * g4, 4 * g4 + 4)
                    ocA = st2.tile([128, 4, W], F32R, tag="ocA", bufs=1)
                    ocB = st2.tile([64, 4, W], F32R, tag="ocB", bufs=1)
                    for msl, omt, npart, bo in (
                            (slice(0, 128), ocA, 128, "boA"),
                            (slice(128, 192), ocB, 64, "boB")):
                        po = ps.tile([128, 512], F32, tag="mm", name="poo")
                        pov = po[0:npart, :].rearrange("p (r w) -> p r w", r=4)
                        nc.tensor.matmul(
                            po[0:npart, :], w["woA"][:, msl],
                            acmA[:, qs, :].rearrange("p a b -> p (a b)"),
                            start=True, stop=False)
                        nc.tensor.matmul(
                            po[0:npart, :], w["woB"][:, msl],
                            acmB[:, qs, :].rearrange("p a b -> p (a b)"),
                            start=False, stop=True)
                        nc.scalar.activation(omt[:], pov, AF.Identity,
                                             bias=w[bo][:], scale=1.0)
                    for j2 in range(2):
                        pt = pso.tile([128, 2, 192], F32R, tag="outt",
                                      bufs=1)
                        for jt in range(2):
                            tt = 2 * j2 + jt
                            nc.tensor.transpose(pt[:, jt, 0:128],
                                                ocA[:, tt, :], w["idnr"][:])
                            nc.tensor.transpose(pt[:, jt, 128:192],
                                                ocB[:, tt, :],
                                                w["idnr"][0:64, 0:64])
                        op_t = st2.tile([W, 2, C], F32, tag="out_pm", bufs=2)
                        nc.scalar.copy(op_t[:], pt[:])
                        t0 = 4 * g4 + 2 * j2
                        nc.sync.dma_start(out_dv[:, h0 + t0:h0 + t0 + 2, :],
                                          op_t[:])

            for p_ in range(npass):
                front_end(0)
                xq_transposes(0)
                for ci in range(NCH):
                    apply_dve(ci, 0, 13)
                    if ci + 1 < NCH:
                        front_end(ci + 1)
                    apply_dve(ci, 13, 25)
                    finish(ci)
                    if ci + 1 < NCH:
                        xq_transposes(ci + 1)

    nc.compile()
    _CACHE[key] = nc
    return nc


def kernel(**inputs):
    from concourse import bass_utils

    nc = build_program()
    wts = _host_weights(inputs)
    x = _bf(np.asarray(inputs["x"], dtype=np.float32))

    in_maps = []
    for core in range(NCORES):
        m = dict(wts)
        m["x"] = np.ascontiguousarray(x[core].reshape(POS, C))
        in_maps.append(m)

    res = bass_utils.run_bass_kernel_spmd(nc, in_maps, list(range(NCORES)))
    out = np.stack([res.results[i]["out"].reshape(H, W, C)
                    for i in range(NCORES)])
    return out


# revision 15
# speedup vs baseline: 1.0271x; 1.0271x over previous
"""Trainium2 Bass kernel for nn_DeformConv2d (B=8, H=W=128, C=192, G=6, K=3).

Data-parallel over batch: one image per NeuronCore (8 cores).

v2: tiny deformable offsets make bilinear sampling an exact 5x5 stencil with
data-dependent per-(position, group) weights S.  Everything heavy runs in
bf16 (5.8e-3 rel err vs the 2e-2 gate).

Per chunk of 16 rows (channel-major front-end, w-major apply):
  - load x (bf16, host-converted), PE-transpose to channel-major
  - xp = x @ w_in (bf16 GEMM, Act adds bias -> bf16, replicate-pad in w)
  - depthwise 3x3 as diagonal bf16 matmuls + fused Act Silu
  - offsets/mask GEMM with host-permuted (tap, group)-ordered columns;
    PE-transpose to w-major; softmax; branch-free bilinear weights;
    27 strided adds build the 25-bin stencil S[w, t, bin, g] bf16
  - S duplicated to channel pairs S2[w, t, bin, g, 2] (SBUF->SBUF DMA) so
    apply multiplies hit DVE 2x_1P (stride-0 group broadcast with a
    unit-stride innermost pair)
  - 5 dx-shifted window PE-transposes of padded xp -> xq[dx] (the window into
    the replicate pad encodes the W-border clamp exactly)
  - apply: acc[w, t, c] += S2[bin] * xq[dx][t+dy]; dx=-2 bin group runs on
    Pool with its own accumulator, rest on DVE; merged at the end
  - transpose acc to channel-major, out = acc @ w_out + b_out, transpose
    back to position-major fp32, DMA out.
"""

import sys

import numpy as np

sys.path.insert(0, "/opt/trn_rl_repo")

B, H, W, C = 8, 128, 128, 192
G, K = 6, 3
K2 = K * K
GC = C // G
OFFSET_SCALE = 0.1
POS = H * W

NCORES = 8
RCH = 16             # rows per chunk
NCH = H // RCH
NBUF = RCH + 4       # buffered rows (+-2 halo)
WP = W + 4           # replicate-padded width for xp
XST = 132            # x_cm row stride (128 data + 4 zero pad)

# bins grouped by dx so each xq[dx] buffer drains in order; dx=-2 group
# (indices 0-4) runs on Pool
BINS = [(dy, dx) for dx in range(-2, 3) for dy in range(-2, 3)]

_CACHE = {}


def _bf(a):
    import ml_dtypes
    return np.ascontiguousarray(np.asarray(a, np.float32)
                                .astype(ml_dtypes.bfloat16))


def _host_weights(inp):
    f = lambda a: np.ascontiguousarray(np.asarray(a, dtype=np.float32))
    w_in = f(inp["w_in"]); b_in = f(inp["b_in"])
    w_dw = f(inp["w_dw"]); b_dw = f(inp["b_dw"])
    w_pw = f(inp["w_pw"]).reshape(C, C); b_pw = f(inp["b_pw"])
    w_off = f(inp["w_off"]); b_off = f(inp["b_off"])
    w_mask = f(inp["w_mask"]); b_mask = f(inp["b_mask"])
    w_out = f(inp["w_out"]); b_out = f(inp["b_out"])

    w_off2 = w_pw @ w_off
    b_off2 = b_pw @ w_off + b_off
    w_msk2 = w_pw @ w_mask
    b_msk2 = b_pw @ w_mask + b_mask

    # permute 54-wide blocks from (g, tap) to (tap, g) order
    perm = np.array([g * K2 + t for t in range(K2) for g in range(G)])
    oh_cols = w_off2[:, 0::2][:, perm] * OFFSET_SCALE
    ow_cols = w_off2[:, 1::2][:, perm] * OFFSET_SCALE
    mk_cols = w_msk2[:, perm]
    b_oh = b_off2[0::2][perm] * OFFSET_SCALE
    b_ow = b_off2[1::2][perm] * OFFSET_SCALE
    b_mk = b_msk2[perm]
    w_om = np.concatenate([oh_cols, ow_cols, mk_cols], axis=1)
    b_om = np.concatenate([b_oh, b_ow, b_mk])

    dwd0 = np.zeros((128, 9, 128), np.float32)
    dwd1 = np.zeros((64, 9, 64), np.float32)
    for k in range(9):
        d = w_dw[k // 3, k % 3, 0, :]
        np.fill_diagonal(dwd0[:, k, :], d[0:128])
        np.fill_diagonal(dwd1[:, k, :], d[128:192])

    return {
        "wiA": _bf(w_in[0:128, :]), "wiB": _bf(w_in[128:192, :]),
        "binA": f(b_in[0:128].reshape(128, 1)),
        "binB": f(b_in[128:192].reshape(64, 1)),
        "dwd0": _bf(dwd0), "dwd1": _bf(dwd1),
        "bdwA": f(b_dw[0:128].reshape(128, 1)),
        "bdwB": f(b_dw[128:192].reshape(64, 1)),
        "womA": _bf(w_om[0:128, :]), "womB": _bf(w_om[128:192, :]),
        "bomA": f(b_om[0:128].reshape(128, 1)),
        "bomB": f(b_om[128:162].reshape(34, 1)),
        "woA": _bf(w_out[0:128, :]), "woB": _bf(w_out[128:192, :]),
        "boA": f(b_out[0:128].reshape(128, 1)),
        "boB": f(b_out[128:192].reshape(64, 1)),
        "idnb": _bf(np.eye(128, dtype=np.float32)),
        "idnr": np.ascontiguousarray(np.eye(128, dtype=np.float32)),
    }


WSHAPES = [
    ("wiA", [128, C], "bf16"), ("wiB", [64, C], "bf16"),
    ("binA", [128, 1], "f32"), ("binB", [64, 1], "f32"),
    ("dwd0", [128, 9, 128], "bf16"), ("dwd1", [64, 9, 64], "bf16"),
    ("bdwA", [128, 1], "f32"), ("bdwB", [64, 1], "f32"),
    ("womA", [128, 162], "bf16"), ("womB", [64, 162], "bf16"),
    ("bomA", [128, 1], "f32"), ("bomB", [34, 1], "f32"),
    ("woA", [128, C], "bf16"), ("woB", [64, C], "bf16"),
    ("boA", [128, 1], "f32"), ("boB", [64, 1], "f32"),
    ("idnb", [128, 128], "bf16"), ("idnr", [128, 128], "f32r"),
]


def build_program(npass=1):
    key = ("nc", npass)
    if key in _CACHE:
        return _CACHE[key]

    import concourse.bacc as bacc
    import concourse.tile as tile
    import concourse.mybir as mybir

    F32 = mybir.dt.float32
    F32R = mybir.dt.float32r
    BF16 = mybir.dt.bfloat16
    OP = mybir.AluOpType
    AF = mybir.ActivationFunctionType
    AX = mybir.AxisListType

    nc = bacc.Bacc(None, target_bir_lowering=False)

    x_d = nc.dram_tensor("x", [POS, C], BF16, kind="ExternalInput")
    out_d = nc.dram_tensor("out", [POS, C], F32, kind="ExternalOutput")
    DT = {"f32": F32, "bf16": BF16, "f32r": F32R}
    wd = {name: nc.dram_tensor(name, shape, DT[dts], kind="ExternalInput")
          for name, shape, dts in WSHAPES}

    x_dv = x_d[:].rearrange("(h p) c -> p h c", p=W)
    out_dv = out_d[:].rearrange("(h p) c -> p h c", p=W)

    with tile.TileContext(nc) as tc:
        with (
            tc.tile_pool(name="wp", bufs=1) as wp,
            tc.tile_pool(name="st1", bufs=1) as st1,
            tc.tile_pool(name="st2", bufs=2) as st2,
            tc.tile_pool(name="ps", bufs=3, space="PSUM") as ps,
            tc.tile_pool(name="psq", bufs=2, space="PSUM") as psq,
            tc.tile_pool(name="pso", bufs=2, space="PSUM") as pso,
        ):
            w = {}
            for name, shape, dts in WSHAPES:
                w[name] = wp.tile(list(shape), DT[dts], tag=name,
                                  name="w_" + name)
                nc.sync.dma_start(w[name][:], wd[name][:])

            state = {}

            def front_end(ci):
                h0 = ci * RCH

                # ---- load + transpose x to channel-major (bf16) ----
                x_cmA = st1.tile([128, NBUF, XST], BF16, tag="x_cmA")
                x_cmB = st1.tile([64, NBUF, XST], BF16, tag="x_cmB")
                for nb in range(5):
                    xt = st2.tile([W, 4, C], BF16, tag="x_pm", bufs=2)
                    rows = [min(max(h0 - 2 + 4 * nb + j, 0), H - 1)
                            for j in range(4)]
                    j = 0
                    while j < 4:
                        j2 = j
                        while j2 + 1 < 4 and rows[j2 + 1] == rows[j2] + 1:
                            j2 += 1
                        nc.sync.dma_start(xt[:, j:j2 + 1, :],
                                          x_dv[:, rows[j]:rows[j2] + 1, :])
                        j = j2 + 1
                    pt = ps.tile([128, 4, 256], BF16, tag="mm", name="ptx")
                    for jr in range(4):
                        nc.tensor.transpose(pt[:, jr, 0:128],
                                            xt[:, jr, 0:128], w["idnb"][:])
                        nc.tensor.transpose(pt[0:64, jr, 128:256],
                                            xt[:, jr, 128:192], w["idnb"][:])
                    r0 = 4 * nb
                    nc.scalar.copy(x_cmA[:, r0:r0 + 4, 0:128],
                                   pt[:, :, 0:128])
                    nc.scalar.copy(x_cmB[:, r0:r0 + 4, 0:128],
                                   pt[0:64, :, 128:256])
                nc.vector.memset(x_cmA[:, :, 128:132], 0.0)
                nc.vector.memset(x_cmB[:, :, 128:132], 0.0)

                # ---- xp = x @ w_in -> padded bf16 buffers ----
                xpA = st1.tile([128, NBUF, WP], BF16, tag="xpA", bufs=2)
                xpB = st1.tile([64, NBUF, WP], BF16, tag="xpB", bufs=2)
                xA_f = x_cmA[:].rearrange("p a b -> p (a b)")
                xB_f = x_cmB[:].rearrange("p a b -> p (a b)")
                ABLK = [(0, 3), (3, 6), (6, 9), (9, 12), (12, 15), (15, 18),
                        (18, 20)]
                for r0, r1 in ABLK:
                    nr = r1 - r0
                    pa = ps.tile([128, 512], F32, tag="mm", name="pa")
                    nc.tensor.matmul(pa[:, 0:XST * nr], w["wiA"][:, 0:128],
                                     xA_f[:, XST * r0:XST * r1],
                                     start=True, stop=False)
                    nc.tensor.matmul(pa[:, 0:XST * nr], w["wiB"][:, 0:128],
                                     xB_f[:, XST * r0:XST * r1],
                                     start=False, stop=True)
                    pav = pa[:, 0:XST * nr].rearrange("p (r w) -> p r w", r=nr)
                    nc.scalar.activation(xpA[:, r0:r1, 2:130],
                                         pav[:, :, 0:128], AF.Identity,
                                         bias=w["binA"][:], scale=1.0)
                    pb = ps.tile([128, 512], F32, tag="mm", name="pb")
                    nc.tensor.matmul(pb[0:64, 0:XST * nr],
                                     w["wiA"][:, 128:192],
                                     xA_f[:, XST * r0:XST * r1],
                                     start=True, stop=False)
                    nc.tensor.matmul(pb[0:64, 0:XST * nr],
                                     w["wiB"][:, 128:192],
                                     xB_f[:, XST * r0:XST * r1],
                                     start=False, stop=True)
                    pbv = pb[0:64, 0:XST * nr].rearrange("p (r w) -> p r w",
                                                         r=nr)
                    nc.scalar.activation(xpB[:, r0:r1, 2:130],
                                         pbv[:, :, 0:128], AF.Identity,
                                         bias=w["binB"][:], scale=1.0)
                for t_, np_ in ((xpA, 128), (xpB, 64)):
                    nc.vector.tensor_copy(
                        t_[:, :, 0:2],
                        t_[:, :, 2:3].broadcast_to([np_, NBUF, 2]))
                    nc.vector.tensor_copy(
                        t_[:, :, 130:132],
                        t_[:, :, 129:130].broadcast_to([np_, NBUF, 2]))

                # ---- depthwise conv + fused SiLU ----
                sA = st1.tile([128, RCH, W], BF16, tag="sA")
                sB = st1.tile([64, RCH, W], BF16, tag="sB")
                taps = [(0, -1), (0, 0), (0, 1), (-1, -1), (-1, 0), (-1, 1),
                        (1, -1), (1, 0), (1, 1)]
                DBLK = [(0, 3), (3, 6), (6, 9), (9, 12), (12, 14), (14, 16)]
                for dwt, cmf, st_, bdw, npart in (
                        ("dwd0", xA_f, sA, "bdwA", 128),
                        ("dwd1", xB_f, sB, "bdwB", 64)):
                    for r0, r1 in DBLK:
                        nr = r1 - r0
                        pd = ps.tile([128, 512], F32, tag="mm", name="pd")
                        issued = 0
                        for ti, (dy, dx) in enumerate(taps):
                            rl, rh_ = r0, r1
                            if ci == 0 and dy == -1:
                                rl = max(rl, 1)
                            if ci == NCH - 1 and dy == 1:
                                rh_ = min(rh_, RCH - 1)
                            if rl >= rh_:
                                continue
                            base = XST * (rl + 2 + dy) + dx
                            nc.tensor.matmul(
                                pd[0:npart, XST * (rl - r0):XST * (rh_ - r0)],
                                w[dwt][:, (dy + 1) * 3 + (dx + 1), :],
                                cmf[:, base:base + XST * (rh_ - rl)],
                                start=(issued == 0),
                                stop=(ti == len(taps) - 1),
                                skip_group_check=True)
                            issued += 1
                        pdv = pd[0:npart, 0:XST * nr].rearrange(
                            "p (r w) -> p r w", r=nr)[:, :, 0:128]
                        nc.scalar.activation(st_[:, r0:r1, :], pdv, AF.Silu,
                                             bias=w[bdw][:], scale=1.0)

                # ---- offsets/mask projection + transpose to w-major ----
                # columns: [oh(54) | ow(54) | mask(54)], each (tap, g)-major
                ohow = st1.tile([W, RCH, 108], BF16, tag="ohow")
                expm = st1.tile([W, RCH, 54], BF16, tag="expm")
                for nb in range(4):
                    rsl = slice(4 * nb, 4 * nb + 4)
                    omA = st2.tile([128, 4, W], BF16, tag="omA", bufs=2)
                    omB = st2.tile([34, 4, W], BF16, tag="omB", bufs=2)
                    for msl, omt, npart, bom in (
                            (slice(0, 128), omA, 128, "bomA"),
                            (slice(128, 162), omB, 34, "bomB")):
                        po = ps.tile([128, 512], F32, tag="mm", name="pom")
                        pov = po[0:npart, :].rearrange("p (r w) -> p r w", r=4)
                        nc.tensor.matmul(
                            po[0:npart, :], w["womA"][:, msl],
                            sA[:, rsl, :].rearrange("p a b -> p (a b)"),
                            start=True, stop=False)
                        nc.tensor.matmul(
                            po[0:npart, :], w["womB"][:, msl],
                            sB[:, rsl, :].rearrange("p a b -> p (a b)"),
                            start=False, stop=True)
                        nc.scalar.activation(omt[:], pov, AF.Identity,
                                             bias=w[bom][:], scale=1.0)
                    pt = psq.tile([128, 4, 162], BF16, tag="tq", name="ptom")
                    for jt in range(4):
                        nc.tensor.transpose(pt[:, jt, 0:128], omA[:, jt, :],
                                            w["idnb"][:])
                        nc.tensor.transpose(pt[:, jt, 128:162], omB[:, jt, :],
                                            w["idnb"][0:34, 0:34])
                    t0 = 4 * nb
                    nc.scalar.copy(ohow[:, t0:t0 + 4, :], pt[:, :, 0:108])
                    nc.scalar.activation(expm[:, t0:t0 + 4, :],
                                         pt[:, :, 108:162], AF.Exp)

                # ---- softmax over taps (tap-major: expm[w, t, 9, 6]) ----
                red = st2.tile([W, RCH, 6], F32, tag="red", bufs=2)
                nc.vector.tensor_reduce(
                    red[:],
                    expm[:].rearrange("p t (k g) -> p t g k", g=6),
                    AX.X, OP.add)
                rec = st2.tile([W, RCH, 6], BF16, tag="rec", bufs=2)
                with nc.allow_low_precision(reason="attn recip in bf16"):
                    nc.vector.reciprocal(rec[:], red[:])
                attn = st1.tile([W, RCH, 54], BF16, tag="attn")
                nc.vector.tensor_tensor(
                    attn[:].rearrange("p t (k g) -> p t k g", g=6),
                    expm[:].rearrange("p t (k g) -> p t k g", g=6),
                    rec[:].unsqueeze(2).broadcast_to([W, RCH, 9, 6]),
                    OP.mult)

                # ---- branch-free bilinear weights ----
                oh_v = ohow[:, :, 0:54]
                ow_v = ohow[:, :, 54:108]
                hwm = st1.tile([W, RCH, 54], BF16, tag="hwm")
                hwp = st1.tile([W, RCH, 54], BF16, tag="hwp")
                wwm = st1.tile([W, RCH, 54], BF16, tag="wwm")
                wwp = st1.tile([W, RCH, 54], BF16, tag="wwp")
                nc.scalar.activation(hwm[:], oh_v, AF.Relu, bias=0.0,
                                     scale=-1.0)
                nc.scalar.activation(hwp[:], oh_v, AF.Relu)
                nc.scalar.activation(wwm[:], ow_v, AF.Relu, bias=0.0,
                                     scale=-1.0)
                nc.scalar.activation(wwp[:], ow_v, AF.Relu)
                # negated centers: |o| - 1 = -(1 - |o|)
                ww0n = st1.tile([W, RCH, 54], BF16, tag="ww0n")
                nc.scalar.activation(ww0n[:], ow_v, AF.Abs)
                nc.vector.tensor_scalar_sub(ww0n[:], ww0n[:], 1.0)
                t1h = st2.tile([W, RCH, 54], BF16, tag="t1h", bufs=2)
                nc.scalar.activation(t1h[:], oh_v, AF.Abs)
                nc.vector.tensor_scalar_sub(t1h[:], t1h[:], 1.0)
                ahm = st1.tile([W, RCH, 54], BF16, tag="ahm")
                ahp = st1.tile([W, RCH, 54], BF16, tag="ahp")
                ah0n = st1.tile([W, RCH, 54], BF16, tag="ah0n")
                nc.vector.tensor_tensor(ahm[:], attn[:], hwm[:], OP.mult)
                nc.vector.tensor_tensor(ahp[:], attn[:], hwp[:], OP.mult)
                nc.vector.tensor_tensor(ah0n[:], attn[:], t1h[:], OP.mult)

                # ---- accumulate 25-bin stencil S[w, t, 25, 6] (bf16) ----
                S = st1.tile([W, 25, RCH, 6], BF16, tag="S")
                nc.vector.memset(S[:], 0.0)
                ah = {-1: (ahm, 1), 0: (ah0n, -1), 1: (ahp, 1)}
                ww = {-1: (wwm, 1), 0: (ww0n, -1), 1: (wwp, 1)}
                for a in (-1, 0, 1):
                    ah_t, sgn_a = ah[a]
                    for b_ in (-1, 0, 1):
                        ww_t, sgn_b = ww[b_]
                        pab = st2.tile([W, RCH, 54], BF16, tag="pab", bufs=3)
                        nc.vector.tensor_tensor(pab[:], ah_t[:], ww_t[:],
                                                OP.mult)
                        op = OP.add if sgn_a * sgn_b > 0 else OP.subtract
                        pv = pab[:].rearrange("p t (rh rw g) -> p rh rw t g",
                                              rh=3, rw=3)
                        for rh_ in range(3):
                            dy5 = rh_ - 1 + a + 2
                            tgt = (S[:]
                                   .rearrange("p (dy dx) t g -> p dy dx t g",
                                              dy=5)
                                   [:, dy5, b_ + 1:b_ + 4, :, :])
                            nc.vector.tensor_tensor(
                                tgt, tgt, pv[:, rh_, :, :, :], op)

                # ---- duplicate S -> S2 channel pairs (SBUF->SBUF DMA) ----
                S2 = st1.tile([W, 25, RCH, 6, 2], BF16, tag="S2", bufs=2)
                nc.vector.tensor_copy(
                    S2[:],
                    S[:].unsqueeze(4).broadcast_to([W, 25, RCH, 6, 2]))

                state[ci] = {"xpA": xpA, "xpB": xpB, "S2": S2, "xq": None, "accd": None}

            def xq_transposes(ci):
                stc = state[ci]
                xpA, xpB = stc["xpA"], stc["xpB"]
                xq = {}
                xq[0] = st1.tile([128, NBUF, C], BF16, tag="xq0",
                                 name="xq0")
                for g4 in range(5):
                    pt = psq.tile([128, 4, 192], BF16, tag="tq", name="ptq")
                    for jr in range(4):
                        r = 4 * g4 + jr
                        nc.tensor.transpose(pt[:, jr, 0:128],
                                            xpA[:, r, 2:130], w["idnb"][:])
                        nc.tensor.transpose(pt[:, jr, 128:192],
                                            xpB[:, r, 2:130],
                                            w["idnb"][0:64, 0:64])
                    if g4 % 2 == 0:
                        nc.vector.tensor_copy(
                            xq[0][:, 4 * g4:4 * g4 + 4, :], pt[:])
                    else:
                        nc.scalar.copy(
                            xq[0][:, 4 * g4:4 * g4 + 4, :], pt[:])
                for dx in (-2, -1, 1, 2):
                    t = st1.tile([128, NBUF, C], BF16, tag=f"xq{dx}",
                                 name=f"xq{dx}")
                    for g4 in range(5):
                        pt = psq.tile([128, 4, 192], BF16, tag="tq",
                                      name="ptq2")
                        for jr in range(4):
                            r = 4 * g4 + jr
                            nc.tensor.transpose(
                                pt[:, jr, 0:128],
                                xpA[:, r, 2 + dx:130 + dx], w["idnb"][:])
                            nc.tensor.transpose(
                                pt[:, jr, 128:192],
                                xpB[:, r, 2 + dx:130 + dx],
                                w["idnb"][0:64, 0:64])
                        if ((dx + 2) * 5 + g4) % 2 == 0:
                            nc.vector.tensor_copy(
                                t[:, 4 * g4:4 * g4 + 4, :], pt[:])
                        else:
                            nc.scalar.copy(
                                t[:, 4 * g4:4 * g4 + 4, :], pt[:])
                    xq[dx] = t
                stc["xq"] = xq

            def bin_views(stc, i):
                dy, dx = BINS[i]
                si = (dy + 2) * 5 + (dx + 2)
                xv = (stc["xq"][dx][:, 2 + dy:2 + dy + RCH, :]
                      .rearrange("p t (g h two) -> p t g h two", g=6, two=2))
                sv = (stc["S2"][:, si, :, :, :].unsqueeze(3)
                      .broadcast_to([W, RCH, 6, GC // 2, 2]))
                return xv, sv


            def apply_dve(ci, lo, hi):
                stc = state[ci]
                accd = stc["accd"]
                if accd is None:
                    accd = st1.tile([128, RCH, C], BF16, tag="accd", bufs=2,
                                    name="accd")
                    stc["accd"] = accd
                for i in range(lo, hi):
                    xv, sv = bin_views(stc, i)
                    if i == 0:
                        av = accd[:].rearrange("p t (g h two) -> p t g h two",
                                               g=6, two=2)
                        nc.vector.tensor_tensor(av, xv, sv, OP.mult)
                    else:
                        tmp = st2.tile([128, RCH, C], BF16, tag="tmpA",
                                       bufs=2, name="tmpA")
                        tv = tmp[:].rearrange("p t (g h two) -> p t g h two",
                                              g=6, two=2)
                        nc.vector.tensor_tensor(tv, xv, sv, OP.mult)
                        nc.vector.tensor_tensor(accd[:], accd[:], tmp[:],
                                                OP.add)

            def finish(ci):
                h0 = ci * RCH
                stc = state.pop(ci)
                accd = stc["accd"]

                # ---- transpose acc to channel-major ----
                acmA = st2.tile([128, RCH, W], BF16, tag="acmA", bufs=1)
                acmB = st2.tile([64, RCH, W], BF16, tag="acmB", bufs=1)
                for g4 in range(4):
                    pc = pso.tile([128, 4, 256], BF16, tag="pc")
                    for jt in range(4):
                        t = 4 * g4 + jt
                        nc.tensor.transpose(pc[:, jt, 0:128],
                                            accd[:, t, 0:128], w["idnb"][:])
                        nc.tensor.transpose(pc[0:64, jt, 128:256],
                                            accd[:, t, 128:192],
                                            w["idnb"][:])
                    t0 = 4 * g4
                    nc.scalar.copy(acmA[:, t0:t0 + 4, :], pc[:, :, 0:128])
                    nc.vector.tensor_copy(acmB[:, t0:t0 + 4, :],
                                          pc[0:64, :, 128:256])

                # ---- out projection + transpose back + store ----
                for g4 in range(4):
                    qs = slice(4 * g4, 4 * g4 + 4)
                    ocA = st2.tile([128, 4, W], F32R, tag="ocA", bufs=1)
                    ocB = st2.tile([64, 4, W], F32R, tag="ocB", bufs=1)
                    for msl, omt, npart, bo in (
                            (slice(0, 128), ocA, 128, "boA"),
                            (slice(128, 192), ocB, 64, "boB")):
                        po = ps.tile([128, 512], F32, tag="mm", name="poo")
                        pov = po[0:npart, :].rearrange("p (r w) -> p r w", r=4)
                        nc.tensor.matmul(
                            po[0:npart, :], w["woA"][:, msl],
                            acmA[:, qs, :].rearrange("p a b -> p (a b)"),
                            start=True, stop=False)
                        nc.tensor.matmul(
                            po[0:npart, :], w["woB"][:, msl],
                            acmB[:, qs, :].rearrange("p a b -> p (a b)"),
                            start=False, stop=True)
                        nc.scalar.activation(omt[:], pov, AF.Identity,
                                             bias=w[bo][:], scale=1.0)
                    for j2 in range(2):
                        pt = pso.tile([128, 2, 192], F32R, tag="outt",
                                      bufs=1)
                        for jt in range(2):
                            tt = 2 * j2 + jt
                            nc.tensor.transpose(pt[:, jt, 0:128],
                                                ocA[:, tt, :], w["idnr"][:])
                            nc.tensor.transpose(pt[:, jt, 128:192],
                                                ocB[:, tt, :],
                                                w["idnr"][0:64, 0:64])
                        op_t = st2.tile([W, 2, C], F32, tag="out_pm", bufs=2)
                        nc.scalar.copy(op_t[:], pt[:])
                        t0 = 4 * g4 + 2 * j2
                        nc.sync.dma_start(out_dv[:, h0 + t0:h0 + t0 + 2, :],
                                          op_t[:])

            for p_ in range(npass):
                front_end(0)
                xq_transposes(0)
                for ci in range(NCH):
                    apply_dve(ci, 0, 13)
                    if ci + 1 < NCH:
                        front_end(ci + 1)
                    apply_dve(ci, 13, 25)
                    finish(ci)
                    if ci + 1 < NCH:
                        xq_transposes(ci + 1)

    nc.compile()
    _CACHE[key] = nc
    return nc


def kernel(**inputs):
    from concourse import bass_utils

    nc = build_program()
    wts = _host_weights(inputs)
    x = _bf(np.asarray(inputs["x"], dtype=np.float32))

    in_maps = []
    for core in range(NCORES):
        m = dict(wts)
        m["x"] = np.ascontiguousarray(x[core].reshape(POS, C))
        in_maps.append(m)

    res = bass_utils.run_bass_kernel_spmd(nc, in_maps, list(range(NCORES)))
    out = np.stack([res.results[i]["out"].reshape(H, W, C)
                    for i in range(NCORES)])
    return out
